# revision 22
# baseline (speedup 1.0000x reference)
"""Trainium2 Bass kernel for nn_AllModel_35828617183965 (prototypical networks).

Self-contained. Host folds BN into conv/fc weights and computes a shifted-
activation bias cascade; the device graph is a single-core Bass/Tile pipeline
(no collectives) run SPMD over 8 NeuronCores, 4 episodes per core.

Device structure (per core, 4 episodes = 1040 image-slots incl. pads):
  - images processed in bursts of 16 on TensorE row-tiled sub-arrays
    (tile_position): conv1 as 4x concurrent 32x128 tiles (4-image block-diag
    weights), conv2 as 8x concurrent 32x64 tiles, conv3 as 2x concurrent
    64x128 tiles; 3 taps per conv accumulate in PSUM with shifted column
    windows (guard columns make padding exact).
  - maxpool+relu+bias fused into ONE scalar_tensor_tensor per PSUM bank:
    out = max(even, odd, -c); the shift constant c cascades into the next
    layer's bias (host-precomputed).
  - AdaptiveAvgPool -> pool_avg; fc/l2norm/prototypes/distances on small
    tiles via TensorE matmuls (incl. ones-matmul partition reductions).
Each episode occupies 13 support bursts (208 slots, 200 real) and 7 query
bursts (112 slots, 100 real); pads duplicate real images and are dropped by
zero one-hot rows / output slicing.  The fixed burst-internal image shuffle
(from the tile packing) is absorbed into the host-built one-hot and a strided
rhs access pattern in the query fc matmul.
"""
import sys
import types
import numpy as np

sys.path.insert(0, '/opt/trn_rl_repo')

# Provide the antenv.axon_hooks registry (absent in this image) so the axon
# boot can register the NTFF profile hook and run_bass_kernel_spmd(trace=True)
# can fetch it.  Must happen before jax initializes the axon platform.
if 'antenv.axon_hooks' not in sys.modules:
    _m = types.ModuleType('antenv.axon_hooks')
    _m._hook = None
    def _set_hook(h, _m=_m):
        _m._hook = h
    def _get_hook(_m=_m):
        return _m._hook
    _m.set_axon_ntff_profile_hook = _set_hook
    _m.get_axon_ntff_profile_hook = _get_hook
    sys.modules['antenv.axon_hooks'] = _m

import ml_dtypes

F16 = np.float16
EPS_BN = 1e-5
N_WAY = 20
B, NS, NQ, C_IN, L0 = 32, 200, 100, 8, 512
NCORES = 8
B_LOC = B // NCORES          # 4 episodes per core
SUP_SLOTS = 208              # 13 bursts per episode
QRY_SLOTS = 112              # 7 bursts per episode
N_SUP = B_LOC * SUP_SLOTS    # 832
N_QRY = B_LOC * QRY_SLOTS    # 448
NSLOT = N_SUP + N_QRY        # 1280
NBURST = NSLOT // 16         # 80


# ---------------- host math ----------------

def _fold_bn(w, b, g, be, m, v):
    scale = g / np.sqrt(v + EPS_BN)
    return w * scale[:, None, None], (b - m) * scale + be


def _prepare(inp):
    f = lambda k: np.asarray(inp[k], dtype=np.float64)
    W1, b1 = _fold_bn(f('w1'), f('b1'), f('g1'), f('be1'), f('m1'), f('v1'))
    W2, b2 = _fold_bn(f('w2'), f('b2'), f('g2'), f('be2'), f('m2'), f('v2'))
    W3, b3 = _fold_bn(f('w3'), f('b3'), f('g3'), f('be3'), f('m3'), f('v3'))
    sf = f('gf') / np.sqrt(f('vf') + EPS_BN)
    wf2 = (f('wf') * sf[:, None]).T              # [K=128, F=128]
    bf2 = (f('bf') - f('mf')) * sf + f('bef')
    c1 = b1
    c2 = b2 + W2.sum(axis=2) @ c1
    c3 = b3 + W3.sum(axis=2) @ c2
    bf3 = bf2 + wf2.T @ c3
    return dict(W1=W1, W2=W2, W3=W3, wf2=wf2, c1=c1, c2=c2, c3=c3, bf3=bf3)


def _layouts(p):
    w1 = np.zeros((128, 3 * 128), np.float64)
    for r in range(4):
        for t in range(3):
            for a in range(4):
                w1[32 * r + 8 * a: 32 * r + 8 * a + 8,
                   128 * t + 32 * a: 128 * t + 32 * a + 32] = p['W1'][:, :, t].T
    w2 = np.zeros((128, 3 * 64), np.float64)
    for s in range(4):
        for t in range(3):
            w2[32 * s: 32 * s + 32, 64 * t: 64 * t + 64] = p['W2'][:, :, t].T
    w3 = np.zeros((128, 3 * 128), np.float64)
    for h in range(2):
        for t in range(3):
            w3[64 * h: 64 * h + 64, 128 * t: 128 * t + 128] = p['W3'][:, :, t].T
    out = dict(w1=w1, w2=w2, w3=w3, wf=p['wf2'] / 64.0)
    out['negb1'] = np.tile(-p['c1'], 4)[:, None]
    out['negc2'] = np.tile(-p['c2'], 2)[:, None]
    out['negc3'] = (-p['c3'])[:, None]
    out['negc2_5'] = np.tile(out['negc2'], (1, 5))
    out['posb1'] = -out['negb1']
    out['posc2'] = -out['negc2']
    out['posc3'] = -out['negc3']
    out['negb1_4'] = np.tile(out['negb1'], (1, 4))
    out['bf3_bcast'] = np.tile(p['bf3'][None, :], (128, 1))
    out['bf3_col'] = p['bf3'][:, None]
    return out


# burst-internal shuffle: means column m holds burst image slot I_OF_M[m]
def _i_of_m():
    out = np.zeros(16, np.int64)
    for m in range(16):
        m3, m2, m1, m0 = (m >> 3) & 1, (m >> 2) & 1, (m >> 1) & 1, m & 1
        out[m] = 8 * m0 + 4 * m2 + 2 * m3 + m1
    return out

I_OF_M = _i_of_m()


def _build_onehot_core(s_lab_core):
    """oh dram layout [128, B_LOC*2*20]: rows = means-column-local index."""
    oh = np.zeros((128, B_LOC * 2 * N_WAY), np.float64)
    for e in range(B_LOC):
        lab = np.asarray(s_lab_core[e])
        onehot = (lab[:, None] == np.arange(N_WAY)[None, :]).astype(np.float64)
        counts = onehot.sum(axis=0)
        counts[counts == 0] = 1.0
        ohn = onehot / counts[None, :]           # [200, 20]
        for c, (off, sz) in enumerate([(0, 128), (128, 80)]):
            blk = np.zeros((128, N_WAY))
            for j in range(sz):
                col = off + j                     # means col within episode
                if col < NS:
                    blk[j] = ohn[col]
            oh[:, (2 * e + c) * N_WAY:(2 * e + c + 1) * N_WAY] = blk
    return oh


def _pad_images(imgs, slots_per_ep, real_per_ep):
    """imgs [B_LOC, real, C, L] -> [B_LOC*slots, C, L] padded by repeating."""
    out = np.empty((B_LOC * slots_per_ep,) + imgs.shape[2:], imgs.dtype)
    for e in range(B_LOC):
        out[e * slots_per_ep: e * slots_per_ep + real_per_ep] = imgs[e]
        out[e * slots_per_ep + real_per_ep: (e + 1) * slots_per_ep] = \
            imgs[e, -1][None]
    return out


# ---------------- device graph ----------------

_CACHE = {}


def _build_nc():
    import concourse.bass as bass
    import concourse.tile as tile
    from concourse import bacc, mybir
    from contextlib import ExitStack
    dt = mybir.dt
    AO = mybir.AluOpType
    AF = mybir.ActivationFunctionType

    nc = bacc.Bacc("TRN2", target_bir_lowering=False, debug=False,
                   num_devices=NCORES)

    dram = {}
    def din(name, shape, dtype):
        dram[name] = nc.dram_tensor(name, list(shape), dtype, kind="ExternalInput")

    din('s_imgs', (N_SUP, C_IN, L0), dt.float16)
    din('q_imgs', (N_QRY, C_IN, L0), dt.float16)
    din('w1', (128, 384), dt.float16)
    din('w2', (128, 192), dt.float16)
    din('w3', (128, 384), dt.float16)
    din('wf', (128, 128), dt.float32)
    din('negb1', (128, 1), dt.float32)
    din('posb1', (128, 1), dt.float32)
    din('posc2', (128, 1), dt.float32)
    din('posc3', (128, 1), dt.float32)
    din('negb1_4', (128, 4), dt.float16)
    din('negc2', (128, 1), dt.float32)
    din('negc3', (128, 1), dt.float32)
    din('negc2_5', (128, 5), dt.float16)
    din('bf3_bcast', (128, 128), dt.float32)
    din('bf3_col', (128, 1), dt.float32)
    din('oh', (128, B_LOC * 2 * N_WAY), dt.float32)
    din('eye', (128, 128), dt.float32)
    din('ones128', (128, 1), dt.float16)
    din('onesrow', (1, 128), dt.float16)
    din('onesrow_f', (1, 128), dt.float32)
    din('ones128_f', (128, 1), dt.float32)
    out_d = nc.dram_tensor('out', [B_LOC, NQ, N_WAY], dt.float32,
                           kind="ExternalOutput")
    import os as _os
    _DBG = bool(int(_os.environ.get('KERNEL_DEBUG', '0')))
    if _DBG:
        dbg_means = nc.dram_tensor('dbg_means', [128, NSLOT], dt.float32,
                                   kind="ExternalOutput")
        dbg_qf = nc.dram_tensor('dbg_qf', [128, QRY_SLOTS], dt.float32,
                                kind="ExternalOutput")
        dbg_pm2 = nc.dram_tensor('dbg_pm2', [128, N_WAY], dt.float32,
                                 kind="ExternalOutput")

    with tile.TileContext(nc) as tc, ExitStack() as ctx:
        cpool = ctx.enter_context(tc.tile_pool(name="consts", bufs=1))
        cs = {}
        for name, shape, d in [
            ('w1', (128, 384), dt.float16), ('w2', (128, 192), dt.float16),
            ('w3', (128, 384), dt.float16), ('wf', (128, 128), dt.float32),
            ('negb1', (128, 1), dt.float32), ('negc2', (128, 1), dt.float32),
            ('posb1', (128, 1), dt.float32), ('posc2', (128, 1), dt.float32),
            ('posc3', (128, 1), dt.float32), ('negb1_4', (128, 4), dt.float16),
            ('negc3', (128, 1), dt.float32), ('negc2_5', (128, 5), dt.float16),
            ('bf3_bcast', (128, 128), dt.float32), ('bf3_col', (128, 1), dt.float32),
            ('oh', (128, B_LOC * 2 * N_WAY), dt.float32),
            ('eye', (128, 128), dt.float32), ('ones128', (128, 1), dt.float16),
            ('onesrow', (1, 128), dt.float16),
            ('onesrow_f', (1, 128), dt.float32), ('ones128_f', (128, 1), dt.float32),
        ]:
            t = cpool.tile(list(shape), d, tag=f"c_{name}")
            nc.sync.dma_start(out=t[:], in_=dram[name].ap())
            cs[name] = t

        means = cpool.tile([128, NSLOT + 8], dt.float32, tag="means")
        dist_stage = cpool.tile([128, B_LOC * N_WAY], dt.float32, tag="dist")

        img_pool = ctx.enter_context(tc.tile_pool(name="img", bufs=4))
        p1_pool = ctx.enter_context(tc.tile_pool(name="p1", bufs=3))
        l3r_pool = ctx.enter_context(tc.tile_pool(name="l3r", bufs=6))
        p3_pool = ctx.enter_context(tc.tile_pool(name="p3", bufs=6))
        tmp_pool = ctx.enter_context(tc.tile_pool(name="tmp", bufs=12))
        ps_pool = ctx.enter_context(tc.tile_pool(name="ps", bufs=4, space="PSUM"))
        ep_pool = ctx.enter_context(tc.tile_pool(name="ep", bufs=4))

        # Software-pipelined emission with a 1-burst skew so the PE
        # instruction stream never waits on the pool chain:
        # iteration b emits  L1(b) -> L3(b-1) -> L2(b).
        st = {}

        def stage_load_l1(b):
            g0 = b * 16
            if g0 < N_SUP:
                src = dram['s_imgs'].ap()[g0:g0 + 16]
            else:
                src = dram['q_imgs'].ap()[g0 - N_SUP:g0 - N_SUP + 16]
            img = img_pool.tile([128, L0 + 2], dt.float16, tag="img")
            nc.gpsimd.memset(img[:, 0:1], 0.0)
            nc.gpsimd.memset(img[:, L0 + 1:L0 + 2], 0.0)
            nc.sync.dma_start(out=img[:, 1:L0 + 1],
                              in_=src.rearrange("i c l -> (i c) l"))
            p1 = p1_pool.tile([128, 4 * 258], dt.float16, tag="pooled1")
            p1v = p1[:].rearrange("p (r c) -> p r c", r=4)
            nc.gpsimd.tensor_copy(out=p1v[:, :, 0], in_=cs['negb1_4'][:, 0:4])
            nc.gpsimd.tensor_copy(out=p1v[:, :, 257], in_=cs['negb1_4'][:, 0:4])
            for rp in range(2):
                ps1 = ps_pool.tile([128, 1024], dt.float32, tag="ps")
                for j in range(2):
                    r = 2 * rp + j
                    for d in range(3):
                        nc.tensor.matmul(
                            out=ps1[:, 512 * j:512 * j + 512],
                            lhsT=cs['w1'][32 * r:32 * r + 32,
                                          128 * d:128 * d + 128],
                            rhs=img[32 * r:32 * r + 32, d:d + 512],
                            start=(d == 0), stop=(d == 2),
                            tile_position=(32 * r, 0))
                v1 = ps1.rearrange("p (j l) -> p j l", j=2)
                tmp1 = tmp_pool.tile([128, 512], dt.float16, tag="tmp")
                t1v = tmp1[:].rearrange("p (j l) -> p j l", j=2)
                nc.scalar.activation(out=t1v[:, :, :], in_=v1[:, :, 0:512:2],
                                     func=AF.Relu, bias=cs['posb1'][:, 0:1])
                nc.vector.scalar_tensor_tensor(
                    out=p1v[:, 2 * rp:2 * rp + 2, 1:257], in0=t1v[:, :, :],
                    scalar=cs['negb1'][:, 0:1], in1=v1[:, :, 1:512:2],
                    op0=AO.add, op1=AO.max)
            st[('p1v', b)] = p1v

        def stage_l2(b):
            p1v = st.pop(('p1v', b))
            T = []
            for ti in range(2):
                tt = l3r_pool.tile([128, 518], dt.float16, tag="l3rhs")
                nc.gpsimd.tensor_copy(out=tt[:, 0:517:129],
                                      in_=cs['negc2_5'][:, 0:5])
                T.append(tt)
            for t2i in range(2):
                p2 = ps_pool.tile([128, 1024], dt.float32, tag="ps")
                for sj in range(2):
                    s = 2 * t2i + sj
                    p1s = p1v[32 * s:32 * s + 32]
                    for c in range(2):
                        for d in range(3):
                            nc.tensor.matmul(
                                out=p2[64 * c:64 * c + 64,
                                       512 * sj:512 * sj + 512],
                                lhsT=cs['w2'][32 * s:32 * s + 32,
                                              64 * d:64 * d + 64],
                                rhs=p1s[:, c:4:2, d:d + 256],
                                start=(d == 0), stop=(d == 2),
                                tile_position=(32 * s, 64 * c))
                src2 = p2.rearrange("p (s b l) -> p s b l", s=2, b=2)
                dst = T[t2i][:, 1:517].rearrange("p (x b l) -> p x b l",
                                                 x=2, b=2)
                tmp2 = tmp_pool.tile([128, 512], dt.float16, tag="tmp")
                t2v = tmp2[:].rearrange("p (s b l) -> p s b l", s=2, b=2)
                nc.scalar.activation(out=t2v[:, :, :, :],
                                     in_=src2[:, :, :, 0:256:2],
                                     func=AF.Relu, bias=cs['posc2'][:, 0:1])
                nc.vector.scalar_tensor_tensor(
                    out=dst[:, :, :, 0:128], in0=t2v[:, :, :, :],
                    scalar=cs['negc2'][:, 0:1], in1=src2[:, :, :, 1:256:2],
                    op0=AO.add, op1=AO.max)
            st[('T', b)] = T

        def stage_l3(b):
            g0 = b * 16
            T = st.pop(('T', b))
            for ti in range(2):
                ps3 = ps_pool.tile([128, 1024], dt.float32, tag="ps")
                for h in range(2):
                    for d in range(3):
                        rhs3 = T[ti][64 * h:64 * h + 64, d:d + 516].rearrange(
                            "p (k l) -> p k l", k=4)[:, :, 0:128]
                        nc.tensor.matmul(
                            out=ps3[:, 512 * h:512 * h + 512],
                            lhsT=cs['w3'][64 * h:64 * h + 64,
                                          128 * d:128 * d + 128],
                            rhs=rhs3,
                            start=(d == 0), stop=(d == 2),
                            tile_position=(64 * h, 0))
                p3 = p3_pool.tile([128, 512], dt.float32, tag="p3")
                s3 = ps3.rearrange("p (h k l) -> p h k l", h=2, k=4)
                d3 = p3.rearrange("p (h k l) -> p h k l", h=2, k=4)
                tmp3 = tmp_pool.tile([128, 512], dt.float16, tag="tmp")
                t3v = tmp3[:, :].rearrange("p (h k l) -> p h k l", h=2, k=4)
                nc.scalar.activation(out=t3v[:, :, :, :],
                                     in_=s3[:, :, :, 0:128:2],
                                     func=AF.Relu, bias=cs['posc3'][:, 0:1])
                nc.vector.scalar_tensor_tensor(
                    out=d3[:, :, :, 0:64], in0=t3v[:, :, :, :],
                    scalar=cs['negc3'][:, 0:1], in1=s3[:, :, :, 1:128:2],
                    op0=AO.add, op1=AO.max)
                for h in range(2):
                    c0 = g0 + 4 * h + 2 * ti
                    mv = means[:, c0:c0 + 16].rearrange("p (b x) -> p b x",
                                                        b=2)
                    nc.vector.tensor_reduce(
                        out=mv[:, :, 0:2].rearrange("p b x -> p x b"),
                        in_=p3[:, 256 * h:256 * h + 256].rearrange(
                            "p (k l) -> p k l", k=4),
                        axis=mybir.AxisListType.X, op=AO.add)

        for b in range(NBURST + 1):
            if b < NBURST:
                stage_load_l1(b)
            if b >= 1:
                stage_l3(b - 1)
            if b < NBURST:
                stage_l2(b)
        # ---------------- epilogue ----------------
        for e in range(B_LOC):
            protoT = ps_pool.tile([128, 512], dt.float32, tag="ps")
            for c, (off, sz) in enumerate([(0, 128), (128, 80)]):
                k0 = e * SUP_SLOTS + off
                fcp = ps_pool.tile([128, 512], dt.float32, tag="ps")
                nc.tensor.matmul(out=fcp[0:sz, 0:128],
                                 lhsT=means[:, k0:k0 + sz],
                                 rhs=cs['wf'][:, 0:128],
                                 start=True, stop=True)
                sf = ep_pool.tile([128, 128], dt.float32, tag="sf")
                nc.vector.tensor_tensor(out=sf[0:sz, :], in0=fcp[0:sz, 0:128],
                                        in1=cs['bf3_bcast'][0:sz, :], op=AO.add)
                nc.vector.tensor_scalar(out=sf[0:sz, :], in0=sf[0:sz, :],
                                        scalar1=0.0, scalar2=None, op0=AO.max)
                sq = ep_pool.tile([128, 128], dt.float32, tag="sq")
                nc.vector.tensor_tensor(out=sq[0:sz, :], in0=sf[0:sz, :],
                                        in1=sf[0:sz, :], op=AO.mult)
                nsq = ep_pool.tile([128, 1], dt.float32, tag="nsq")
                nc.vector.tensor_reduce(out=nsq[0:sz, :], in_=sq[0:sz, :],
                                        axis=mybir.AxisListType.X, op=AO.add)
                nrm = ep_pool.tile([128, 1], dt.float32, tag="nrm")
                nc.scalar.activation(out=nrm[0:sz, :], in_=nsq[0:sz, :],
                                     func=AF.Sqrt)
                inv = ep_pool.tile([128, 1], dt.float32, tag="inv")
                nc.vector.reciprocal(out=inv[0:sz, :], in_=nrm[0:sz, :])
                sfn = ep_pool.tile([128, 128], dt.float32, tag="sfn")
                nc.vector.tensor_scalar(out=sfn[0:sz, :], in0=sf[0:sz, :],
                                        scalar1=inv[0:sz, 0:1], scalar2=None,
                                        op0=AO.mult)
                nc.tensor.matmul(
                    out=protoT[0:N_WAY, 0:128],
                    lhsT=cs['oh'][0:sz,
                                  (2 * e + c) * N_WAY:(2 * e + c + 1) * N_WAY],
                    rhs=sfn[0:sz, 0:128],
                    start=(c == 0), stop=(c == 1))
            ptm2 = ep_pool.tile([N_WAY, 128], dt.float32, tag="ptm2")
            nc.vector.tensor_scalar(out=ptm2[:, :], in0=protoT[0:N_WAY, 0:128],
                                    scalar1=-2.0, scalar2=None, op0=AO.mult)
            trp = ps_pool.tile([128, 512], dt.float32, tag="ps")
            nc.tensor.transpose(out=trp[0:128, 0:N_WAY], in_=ptm2[:, :],
                                identity=cs['eye'][0:N_WAY, 0:N_WAY])
            pm2 = ep_pool.tile([128, N_WAY], dt.float32, tag="pm2")
            nc.vector.tensor_copy(out=pm2[:, :], in_=trp[0:128, 0:N_WAY])
            pm2sq = ep_pool.tile([128, N_WAY], dt.float32, tag="pm2sq")
            nc.vector.tensor_tensor(out=pm2sq[:, :], in0=pm2[:, :],
                                    in1=pm2[:, :], op=AO.mult)
            pnp = ps_pool.tile([128, 512], dt.float32, tag="ps")
            nc.tensor.matmul(out=pnp[0:1, 0:N_WAY], lhsT=cs['ones128_f'][:, 0:1],
                             rhs=pm2sq[:, :], start=True, stop=True)
            row = ep_pool.tile([1, N_WAY], dt.float32, tag="row")
            nc.vector.tensor_scalar(out=row[:, :], in0=pnp[0:1, 0:N_WAY],
                                    scalar1=0.25, scalar2=1.0,
                                    op0=AO.mult, op1=AO.add)

            # queries: unpermute via strided rhs APs, one MM per burst
            qbase = N_SUP + e * QRY_SLOTS
            qp = ps_pool.tile([128, 512], dt.float32, tag="ps")
            nc.tensor.matmul(out=qp[0:128, 0:QRY_SLOTS],
                             lhsT=cs['wf'][:, 0:128],
                             rhs=means[:, qbase:qbase + QRY_SLOTS],
                             start=True, stop=True)
            qf = ep_pool.tile([128, QRY_SLOTS], dt.float32, tag="qf")
            nc.scalar.activation(out=qf[:, :], in_=qp[0:128, 0:QRY_SLOTS],
                                 func=AF.Relu, bias=cs['bf3_col'][:, 0:1])
            qsq = ep_pool.tile([128, QRY_SLOTS], dt.float32, tag="qsq")
            nc.vector.tensor_tensor(out=qsq[:, :], in0=qf[:, :], in1=qf[:, :],
                                    op=AO.mult)
            nqp = ps_pool.tile([128, 512], dt.float32, tag="ps")
            nc.tensor.matmul(out=nqp[0:1, 0:QRY_SLOTS],
                             lhsT=cs['ones128_f'][:, 0:1],
                             rhs=qsq[:, :], start=True, stop=True)
            nrmq = ep_pool.tile([1, QRY_SLOTS], dt.float32, tag="nrmq")
            nc.scalar.activation(out=nrmq[:, :], in_=nqp[0:1, 0:QRY_SLOTS],
                                 func=AF.Sqrt)
            invq_f = ep_pool.tile([1, QRY_SLOTS], dt.float32, tag="invqf")
            nc.vector.reciprocal(out=invq_f[:, :], in_=nrmq[:, :])
            ivb = ps_pool.tile([128, 512], dt.float32, tag="ps")
            nc.tensor.matmul(out=ivb[0:128, 0:QRY_SLOTS],
                             lhsT=cs['onesrow_f'][0:1, :],
                             rhs=invq_f[:, :], start=True, stop=True)
            qfn = ep_pool.tile([128, QRY_SLOTS], dt.float32, tag="qfn")
            nc.vector.tensor_tensor(out=qfn[:, :], in0=qf[:, :],
                                    in1=ivb[0:128, 0:QRY_SLOTS], op=AO.mult)
            if _DBG and e == 0:
                _dbg_qf_t = cpool.tile([128, QRY_SLOTS], dt.float32, tag="dbgqf")
                nc.vector.tensor_copy(out=_dbg_qf_t[:, :], in_=qfn[:, :])
                _dbg_pm2_t = cpool.tile([128, N_WAY], dt.float32, tag="dbgpm2")
                nc.vector.tensor_copy(out=_dbg_pm2_t[:, :], in_=pm2[:, :])
            dp = ps_pool.tile([128, 512], dt.float32, tag="ps")
            nc.tensor.matmul(out=dp[0:QRY_SLOTS, 0:N_WAY], lhsT=qfn[:, :],
                             rhs=pm2[:, :], start=True, stop=False)
            nc.tensor.matmul(out=dp[0:QRY_SLOTS, 0:N_WAY],
                             lhsT=cs['onesrow_f'][0:1, 0:QRY_SLOTS],
                             rhs=row[:, :], start=False, stop=True)
            dc = ep_pool.tile([128, N_WAY], dt.float32, tag="dc")
            nc.vector.tensor_scalar(out=dc[0:NQ, :], in0=dp[0:NQ, 0:N_WAY],
                                    scalar1=0.0, scalar2=None, op0=AO.max)
            dsq = ep_pool.tile([128, N_WAY], dt.float32, tag="dsq")
            nc.scalar.activation(out=dsq[0:NQ, :], in_=dc[0:NQ, :],
                                 func=AF.Sqrt)
            nc.vector.tensor_scalar(
                out=dist_stage[0:NQ, e * N_WAY:(e + 1) * N_WAY],
                in0=dsq[0:NQ, :], scalar1=-1.0, scalar2=None, op0=AO.mult)

        nc.sync.dma_start(
            out=out_d.ap().rearrange("e q w -> q e w"),
            in_=dist_stage[0:NQ, :].rearrange("q (e w) -> q e w", e=B_LOC))

        if _DBG:
            nc.sync.dma_start(out=dbg_means.ap(), in_=means[:, :])
            nc.sync.dma_start(out=dbg_qf.ap(), in_=_dbg_qf_t[:, :])
            nc.sync.dma_start(out=dbg_pm2.ap(), in_=_dbg_pm2_t[:, :])

    nc.compile()
    return nc


def _host_inputs(inputs):
    p = _prepare(inputs)
    lay = _layouts(p)
    f32 = lambda a: np.ascontiguousarray(a, dtype=np.float32)
    b16 = lambda a: np.ascontiguousarray(np.asarray(a, np.float32).astype(F16))
    s_img = np.asarray(inputs['s_img'], np.float32)
    q_img = np.asarray(inputs['q_img'], np.float32)
    s_lab = np.asarray(inputs['s_lab'])
    common = {
        'w1': b16(lay['w1']), 'w2': b16(lay['w2']), 'w3': b16(lay['w3']),
        'wf': f32(lay['wf']),
        'negb1': f32(lay['negb1']), 'negc2': f32(lay['negc2']),
        'posb1': f32(lay['posb1']), 'posc2': f32(lay['posc2']),
        'posc3': f32(lay['posc3']), 'negb1_4': b16(lay['negb1_4']),
        'negc3': f32(lay['negc3']), 'negc2_5': b16(lay['negc2_5']),
        'bf3_bcast': f32(lay['bf3_bcast']), 'bf3_col': f32(lay['bf3_col']),
        'eye': f32(np.eye(128)), 'ones128': b16(np.ones((128, 1))),
        'onesrow': b16(np.ones((1, 128))),
        'onesrow_f': f32(np.ones((1, 128))), 'ones128_f': f32(np.ones((128, 1))),
    }
    in_maps = []
    for i in range(NCORES):
        e0 = i * B_LOC
        m = dict(common)
        m['s_imgs'] = b16(_pad_images(s_img[e0:e0 + B_LOC], SUP_SLOTS, NS))
        m['q_imgs'] = b16(_pad_images(q_img[e0:e0 + B_LOC], QRY_SLOTS, NQ))
        m['oh'] = f32(_build_onehot_core(s_lab[e0:e0 + B_LOC]))
        in_maps.append(m)
    return in_maps


def _ensure_ntff_hook():
    try:
        from antenv.axon_hooks import (get_axon_ntff_profile_hook,
                                       set_axon_ntff_profile_hook)
        if get_axon_ntff_profile_hook() is None:
            from trn_agent_boot.trn_boot import _ntff_profile_via_ctypes
            set_axon_ntff_profile_hook(
                _ntff_profile_via_ctypes('/opt/axon/libaxon_pjrt.so'))
    except Exception as e:
        print('ntff hook setup failed:', e)


def _run(inputs, trace=False):
    from concourse.bass_utils import run_bass_kernel_spmd
    if trace:
        _ensure_ntff_hook()
    if 'nc' not in _CACHE:
        _CACHE['nc'] = _build_nc()
    nc = _CACHE['nc']
    in_maps = _host_inputs(inputs)
    res = run_bass_kernel_spmd(nc, in_maps, core_ids=list(range(NCORES)),
                               trace=trace)
    outs = [np.asarray(res.results[i]['out']) for i in range(NCORES)]
    full = np.concatenate(outs, axis=0).astype(np.float32)
    return full, res


def kernel(**inputs):
    out, _ = _run(inputs, trace=False)
    return out


def run_traced(**inputs):
    return _run(inputs, trace=True)
